# revision 9
# baseline (speedup 1.0000x reference)
"""Trainium2 Bass kernel for an attention block (B=16, C=512, T=2048).

reference:
  q = wq@x + bq; k = wk@x + bk; v = wv@x + bv          (conv1x1 per sample)
  attn = softmax(q^T k over s); out = v @ attn^T
  result = gamma * out + x

Sharding: data-parallel over batch across 8 NeuronCores (2 samples/core),
weights replicated.

Device algorithm (v2):
  - host folds gamma into wv, and gamma*bv + x into the residual xg
    (softmax rows sum to 1, so the v-bias is a per-channel constant);
    bk is dropped (a per-t constant in scores cancels in softmax over s).
  - q/k/scores path in fp16; v/softmax-weights path in bf16 (range:
    exp(S) reaches ~e^64); PSUM accumulation always fp32.
  - head: weights land via the gpsimd queue, x[0] sliced on the sync
    queue, x[1] on the scalar queue (parallel DMA); a short warmup
    matmul burst on `ones` flips the HAM clock gate to 8/8 early.
  - phase 1 (both samples): v^T[s,o] tiles via matmul(lhsT=x[c,s],
    rhs=(g*wv)^T[c,o]) into one [128,16,512] tile per sample; q/k via
    one M=128 matmul (k rows 0:64, q rows 64:128 with bias; q DMA-shifted
    to partitions 0:64 so S^T operands can row-pack the PE).
  - phase 2: 9 steps (7x 512-wide t-chunks + 2x 256 at the tail so the
    last finals chain is short). Per step, per pair of 128-wide s-chunks
    (one S^T/exp pair emitted ahead, crossing step boundaries):
      S^T halves into a 3-deep PSUM ring (fp16 row-packed matmuls)
      E = exp(S^T)       (ACT, per-half, PSUM -> SBUF bf16)
      E2 = Ea + Eb       (GpSimd)
      out[cc] += matmul(lhsT=v^T[s,c], rhs=E)   (bf16, 4 c-chunks)
    den work is an add-tree (E4/E8/E16 on DVE, bf16) + ONE ones-matmul
    per step, emitted inside the NEXT step so its operands are ready.
  - finals for step i run inside step i+1: oacc -> SBUF as bf16 (DVE,
    emitted before the tree tail so banks free in time), r = exp(-ln(den))
    on ACT (same table set as Exp; DVE reciprocal costs 4us),
    result = out0*r + xg -> one DMA out. Only the last (256-wide) step's
    finals are exposed; they split muls across DVE and GpSimd.
"""
import numpy as np
import ml_dtypes
import concourse.bass as bass
import concourse.bacc as bacc
import concourse.tile as tile
from concourse import mybir
from concourse.bass_utils import run_bass_kernel_spmd

F32 = mybir.dt.float32
FP16 = mybir.dt.float16
BF16 = mybir.dt.bfloat16
AF = mybir.ActivationFunctionType

B, C, T, D = 16, 512, 2048, 64
NCORES = 8
BPC = B // NCORES          # samples per core
CCH = C // 128             # 4 channel chunks
SCH = T // 128             # 16 s chunks
NPR = SCH // 2             # 8 s-chunk pairs

# phase-2 steps: (sample, t-offset, t-width)
STEPS = ([(0, tc * 512, 512) for tc in range(4)]
         + [(1, tc * 512, 512) for tc in range(3)]
         + [(1, 1536, 256), (1, 1792, 128), (1, 1920, 128)])

PROFILE = False            # set True before calling kernel() to capture HW time
LAST_EXEC_NS = None
_CACHE = {}


def _build():
    nc = bacc.Bacc("TRN2", target_bir_lowering=False, debug=False,
                   enable_asserts=False)
    xd = nc.dram_tensor("x", [BPC, 128, CCH, T], FP16,
                        kind="ExternalInput").ap()
    xgd = nc.dram_tensor("xg", [BPC, 128, CCH, T], F32,
                         kind="ExternalInput").ap()
    wkqT = nc.dram_tensor("wkqT", [C, 2 * D], FP16, kind="ExternalInput").ap()
    wvT = nc.dram_tensor("wvT", [C, C], FP16, kind="ExternalInput").ap()
    bqd = nc.dram_tensor("bq", [D, 1], F32, kind="ExternalInput").ap()
    onesd = nc.dram_tensor("ones", [128, 128], BF16, kind="ExternalInput").ap()
    m30d = nc.dram_tensor("m30", [128, 1], F32, kind="ExternalInput").ap()
    outd = nc.dram_tensor("out", [BPC, 128, CCH, T], F32,
                          kind="ExternalOutput").ap()

    with tile.TileContext(nc) as tc:
        with tc.tile_pool(name="const", bufs=1) as constp, \
             tc.tile_pool(name="xp", bufs=1) as xp, \
             tc.tile_pool(name="vtp", bufs=1) as vtp, \
             tc.tile_pool(name="qkp", bufs=1) as qkp, \
             tc.tile_pool(name="etp", bufs=1) as etp, \
             tc.tile_pool(name="finp", bufs=1) as finp, \
             tc.tile_pool(name="ps", bufs=1, space="PSUM") as ps:

            # ---- warmup source: memset (no DMA dependency), so the PE
            # can start spinning at t~0 while everything streams in
            warm_src = constp.tile([128, 128], BF16, name="wsrc", tag="wsrc")
            nc.gpsimd.memset(warm_src[:], 1.0)

            # ---- constants: all on the gpsimd queue (sync carries x)
            wv_big = constp.tile([128, CCH, C], FP16)
            nc.gpsimd.dma_start(
                out=wv_big, in_=wvT.rearrange("(c p) o -> p c o", p=128))
            wkq_big = constp.tile([128, CCH, 2 * D], FP16)
            nc.gpsimd.dma_start(
                out=wkq_big, in_=wkqT.rearrange("(c p) d -> p c d", p=128))
            ones = constp.tile([128, 128], BF16)
            nc.gpsimd.dma_start(out=ones, in_=onesd)
            bq_full = constp.tile([128, 1], F32)
            nc.gpsimd.dma_start(out=bq_full[D:2 * D, :], in_=bqd)
            bq_hi = bq_full[D:2 * D, :]
            m30 = constp.tile([128, 1], F32)
            nc.gpsimd.dma_start(out=m30, in_=m30d)

            # ---- x loads: sample 0 sliced on sync, sample 1 on scalar
            x_big_all = [xp.tile([128, CCH, T], FP16, name=f"x_{b}",
                                 tag=f"x{b}") for b in range(BPC)]
            x0_slices = [(0, 256), (256, 256), (512, 512), (1024, 512),
                         (1536, 512)]
            for off, w in x0_slices:
                nc.sync.dma_start(out=x_big_all[0][:, :, off:off + w],
                                  in_=xd[0, :, :, off:off + w])
            for q2 in range(2):
                qsl = slice(q2 * 1024, (q2 + 1) * 1024)
                nc.scalar.dma_start(out=x_big_all[1][:, :, qsl],
                                    in_=xd[1, :, :, qsl])
            wv_sb = [wv_big[:, cc, :] for cc in range(CCH)]
            wkq_sb = [wkq_big[:, cc, :] for cc in range(CCH)]
            x_sb_all = [[x_big_all[b][:, cc, :] for cc in range(CCH)]
                        for b in range(BPC)]

            # ---- warmup: keep the PE busy while x streams in, so the
            # HAM clock gate opens (~3.4us of activity) before real work
            warm = ps.tile([128, 512], F32, name="warm", tag="stp", bufs=3)
            for i in range(24):
                nc.tensor.matmul(warm[:, 0:128], warm_src, warm_src,
                                 start=(i == 0), stop=(i == 23))

            # ================= phase 1: v^T and q/k, both samples ========
            vt_all, q_all, k_all = {}, {}, {}
            qhi_all, khi_all = {}, {}
            et = {}       # (si, pr) -> (e_a, e_b)
            e2s = {}      # (si, pr) -> e2 tile

            def emit_st2(si, pr):
                b, toff, w = STEPS[si]
                halves = []
                for h in range(2):
                    sc = 2 * pr + h
                    stp_h = ps.tile([128, 512], F32,
                                    name=f"st_{si}_{pr}_{h}", tag="stp",
                                    bufs=3)
                    if h == 0:
                        lhsT = k_all[b][:, sc * 128:(sc + 1) * 128]
                        rhs = q_all[b][:, toff:toff + w]
                    else:
                        lhsT = khi_all[b][D:2 * D, sc * 128:(sc + 1) * 128]
                        rhs = qhi_all[b][D:2 * D, toff:toff + w]
                    nc.tensor.matmul(stp_h[:, :w], lhsT, rhs,
                                     start=True, stop=True)
                    halves.append(stp_h)
                es = []
                for h in range(2):
                    e_h = etp.tile([128, 512], BF16,
                                   name=f"et_{si}_{pr}_{h}", tag="et",
                                   bufs=4)
                    # constant shift cancels in softmax; keeps den in a
                    # range where the ACT Ln/Exp splines are well-behaved
                    nc.scalar.activation(out=e_h[:, :w],
                                         in_=halves[h][:, :w], func=AF.Exp,
                                         bias=m30[:])
                    es.append(e_h)
                et[(si, pr)] = tuple(es)

            for b in range(BPC):
                x_sb = x_sb_all[b]

                # v^T tiles (bf16): vt[b][:, sc, o]
                vt_big = vtp.tile([128, SCH, C], BF16, name=f"vt_{b}",
                                  tag=f"vt{b}")
                for sc in range(SCH):
                    vps = ps.tile([128, 512], F32, name=f"vps_{b}_{sc}",
                                  tag=f"o{sc % 2}")
                    for cc in range(CCH):
                        nc.tensor.matmul(
                            vps[:], x_sb[cc][:, sc * 128:(sc + 1) * 128],
                            wv_sb[cc][:],
                            start=(cc == 0), stop=(cc == CCH - 1))
                    nc.vector.tensor_copy(out=vt_big[:, sc, :], in_=vps[:])
                vt_all[b] = vt_big

                # q, k via one M=128 matmul; q shifted to partitions 0:64,
                # k replicated to 64:128 so S^T pairs can row-pack the PE
                q_hi = qkp.tile([128, T], FP16, name=f"qh_{b}", tag=f"qh{b}")
                k_hi = qkp.tile([128, T], FP16, name=f"kh_{b}", tag=f"kh{b}")
                q_sb = qkp.tile([D, T], FP16, name=f"q_{b}", tag=f"q{b}")
                k_sb = qkp.tile([D, T], FP16, name=f"k_{b}", tag=f"k{b}")
                for tc_i in range(4):
                    tsl = slice(tc_i * 512, (tc_i + 1) * 512)
                    qps = ps.tile([128, 512], F32, name=f"qps_{b}_{tc_i}",
                                  tag=f"o{2 + tc_i % 2}")
                    for cc in range(CCH):
                        nc.tensor.matmul(qps[:], wkq_sb[cc][:],
                                         x_sb[cc][:, tsl],
                                         start=(cc == 0), stop=(cc == CCH - 1))
                    nc.vector.tensor_copy(out=k_sb[:, tsl], in_=qps[0:D, :])
                    nc.scalar.activation(out=q_hi[D:2 * D, tsl],
                                         in_=qps[D:2 * D, :],
                                         func=AF.Identity, bias=bq_hi[:],
                                         scale=1.0)
                    nc.gpsimd.dma_start(out=q_sb[:, tsl],
                                        in_=q_hi[D:2 * D, tsl])
                nc.gpsimd.dma_start(out=k_hi[D:2 * D, :], in_=k_sb[:, :])
                q_all[b], k_all[b] = q_sb, k_sb
                qhi_all[b], khi_all[b] = q_hi, k_hi

                if b == 0:
                    # first S^T/exp pair warms up under sample 1's prework
                    emit_st2(0, 0)

            # ================= phase 2: attention, all steps =============
            nsteps = len(STEPS)
            # deferred state from the previous step, resolved inside the
            # current one: den matmul (pr=1), finals (pr=2)
            pend_den = None      # (si, e16)
            pend_fin = None      # (si, den_ps, ob, xg_t)

            def emit_den(si, e16):
                b, toff, w = STEPS[si]
                den_ps = ps.tile([128, 512], F32, name=f"den_{si}",
                                 tag="den", bufs=1)
                nc.tensor.matmul(den_ps[:, :w], ones, e16[:, :w],
                                 start=True, stop=True)
                return den_ps

            def emit_finals(si, den_ps, ob, xg_t, last=False):
                b, toff, w = STEPS[si]
                scr = finp.tile([128, 512], F32, name=f"sc_{si}", tag="scr",
                                bufs=2)
                recip = finp.tile([128, 512], F32, name=f"rc_{si}", tag="rc",
                                  bufs=2)
                nc.vector.reciprocal_approx_accurate(
                    out=recip[:, :w], in_=den_ps[:, :w], scratch=scr[:, :w])
                t_f = finp.tile([128, CCH, 512], F32, name=f"f_{si}",
                                tag="f", bufs=2)
                for cc in range(CCH):
                    eng = nc.gpsimd if (last and cc >= 2) else nc.vector
                    eng.tensor_mul(t_f[:, cc, :w], ob[:, cc, :w],
                                   recip[:, :w])
                    eng.tensor_add(t_f[:, cc, :w], t_f[:, cc, :w],
                                   xg_t[:, cc, :w])
                    if last:
                        nc.sync.dma_start(out=outd[b, :, cc, toff:toff + w],
                                          in_=t_f[:, cc, :w])
                if not last:
                    nc.sync.dma_start(out=outd[b, :, :, toff:toff + w],
                                      in_=t_f[:, :, :w])

            for si in range(nsteps):
                b, toff, w = STEPS[si]
                oacc = [ps.tile([128, 512], F32, name=f"o_{si}_{cc}",
                                tag=f"o{cc}") for cc in range(CCH)]
                xg_t = finp.tile([128, CCH, 512], F32,
                                 name=f"xg_{si}", tag="xg", bufs=3)
                nc.sync.dma_start(out=xg_t[:, :, :w],
                                  in_=xgd[b, :, :, toff:toff + w])
                e4 = []   # per 2 pairs
                e8 = []
                ob = finp.tile([128, CCH, 512], BF16, name=f"ob_{si}",
                               tag="ob", bufs=2)

                for pr in range(NPR):
                    # keep one S^T/exp pair in flight ahead of the consumers
                    if pr + 1 < NPR:
                        emit_st2(si, pr + 1)
                    elif si + 1 < nsteps:
                        emit_st2(si + 1, 0)
                    e_a, e_b = et.pop((si, pr))
                    e2 = etp.tile([128, 512], BF16, name=f"e2_{si}_{pr}",
                                  tag="e2", bufs=3)
                    nc.gpsimd.tensor_add(e2[:, :w], e_a[:, :w], e_b[:, :w])
                    e2s[(si, pr)] = e2
                    # out accumulation; the step's first pair staggers the
                    # oacc first-touch order so the previous step's bank
                    # copies (DVE) have time to land
                    order = [(h, cc) for h in range(2)
                             for cc in range(CCH)]
                    for h, cc in order:
                        sc = 2 * pr + h
                        e_sl = (e_a if h == 0 else e_b)[:, :w]
                        nc.tensor.matmul(
                            oacc[cc][:, :w],
                            vt_all[b][:, sc, cc * 128:(cc + 1) * 128],
                            e_sl, start=(sc == 0), stop=(sc == SCH - 1))
                        if sc == SCH - 1:
                            # bank-freeing copy right behind the last
                            # accumulating matmul so DVE starts it early
                            with tc.high_priority():
                                nc.vector.tensor_copy(out=ob[:, cc, :w],
                                                      in_=oacc[cc][:, :w])
                    # deferred work from the previous step + den add-tree
                    # (tree runs on gpsimd so the DVE queue stays clear for
                    # the bank-freeing copies at the step boundary; the
                    # scheduler orders by estimated readiness, not program
                    # order, so engine choice is the only reliable lever)
                    treng = nc.vector if si == nsteps - 1 else nc.gpsimd
                    if pr == 2 and pend_den is not None:
                        dsi, de16 = pend_den
                        dden = emit_den(dsi, de16)
                        pend_fin = (dsi, dden, *pend_fin_args)
                        pend_den = None
                    if pr == 3 and pend_fin is not None:
                        fsi, fden, fob, fxg = pend_fin
                        emit_finals(fsi, fden, fob, fxg)
                        pend_fin = None
                    if pr % 2 == 1:
                        t4 = etp.tile([128, 512], BF16,
                                      name=f"e4_{si}_{pr // 2}", tag="e4",
                                      bufs=2)
                        treng.tensor_add(t4[:, :w],
                                         e2s.pop((si, pr - 1))[:, :w],
                                         e2s.pop((si, pr))[:, :w])
                        e4.append(t4)
                    if pr == 3 or pr == 7:
                        t8 = etp.tile([128, 512], BF16,
                                      name=f"e8_{si}_{pr // 4}", tag="e8",
                                      bufs=2)
                        treng.tensor_add(t8[:, :w], e4[-2][:, :w],
                                         e4[-1][:, :w])
                        e8.append(t8)

                # end of step: bank-freeing copies (bf16) first, then the
                # tree tail; den matmul + finals are deferred into step si+1
                last = si == nsteps - 1
                e16 = etp.tile([128, 512], BF16, name=f"e16_{si}", tag="e16",
                               bufs=2)
                treng.tensor_add(e16[:, :w], e8[-2][:, :w], e8[-1][:, :w])
                if last:
                    den_ps = emit_den(si, e16)
                    emit_finals(si, den_ps, ob, xg_t, last=True)
                else:
                    pend_den = (si, e16)
                    pend_fin_args = (ob, xg_t)
    nc.compile()
    return nc


def _get_nc():
    if "nc" not in _CACHE:
        _CACHE["nc"] = _build()
    return _CACHE["nc"]


def kernel(x, wq, bq, wk, bk, wv, bv, gamma):
    global LAST_EXEC_NS
    g = float(np.asarray(gamma).reshape(-1)[0])
    x = np.asarray(x, np.float32)
    # fold gamma into the v path; bk cancels inside softmax; the v bias
    # contributes gamma*bv per channel (softmax rows sum to 1) -> fold it
    # plus the residual into xg
    wvT = np.ascontiguousarray(
        (g * np.asarray(wv, np.float32)).T).astype(np.float16)
    wkqT = np.concatenate([np.asarray(wk, np.float32).T,
                           np.asarray(wq, np.float32).T],
                          axis=1).astype(np.float16)
    bq2 = np.asarray(bq, np.float32).reshape(D, 1)
    gbv = (g * np.asarray(bv, np.float32)).reshape(1, C, 1)
    xg = x + gbv
    # device layout: [B, p, cc, T] with original c == cc*128 + p
    xg = np.ascontiguousarray(
        xg.reshape(B, CCH, 128, T).transpose(0, 2, 1, 3))
    ones = np.ones((128, 128), ml_dtypes.bfloat16)
    m30c = np.full((128, 1), -30.0, np.float32)
    xh = np.ascontiguousarray(
        x.reshape(B, CCH, 128, T).transpose(0, 2, 1, 3)).astype(np.float16)

    in_maps = []
    for core in range(NCORES):
        sl = slice(core * BPC, (core + 1) * BPC)
        in_maps.append({
            "x": xh[sl], "xg": xg[sl],
            "wkqT": wkqT, "wvT": wvT,
            "bq": bq2, "ones": ones, "m30": m30c,
        })

    nc = _get_nc()
    res = run_bass_kernel_spmd(nc, in_maps, core_ids=list(range(NCORES)),
                               trace=PROFILE)
    LAST_EXEC_NS = res.exec_time_ns
    out = np.empty((B, 128, CCH, T), np.float32)
    for core in range(NCORES):
        out[core * BPC:(core + 1) * BPC] = res.results[core]["out"]
    return np.ascontiguousarray(
        out.transpose(0, 2, 1, 3)).reshape(B, C, T)


# revision 10
# speedup vs baseline: 1.2193x; 1.2193x over previous
"""Trainium2 Bass kernel for an attention block (B=16, C=512, T=2048).

reference:
  q = wq@x + bq; k = wk@x + bk; v = wv@x + bv          (conv1x1 per sample)
  attn = softmax(q^T k over s); out = v @ attn^T
  result = gamma * out + x

Sharding: data-parallel over batch across 8 NeuronCores (2 samples/core),
weights replicated.

Device algorithm (v2):
  - host folds gamma into wv, and gamma*bv + x into the residual xg
    (softmax rows sum to 1, so the v-bias is a per-channel constant);
    bk is dropped (a per-t constant in scores cancels in softmax over s).
  - q/k/scores path in fp16; v/softmax-weights path in bf16 (range:
    exp(S) reaches ~e^64); PSUM accumulation always fp32.
  - head: weights land via the gpsimd queue, x[0] sliced on the sync
    queue, x[1] on the scalar queue (parallel DMA); a short warmup
    matmul burst on `ones` flips the HAM clock gate to 8/8 early.
  - phase 1 (both samples): v^T[s,o] tiles via matmul(lhsT=x[c,s],
    rhs=(g*wv)^T[c,o]) into one [128,16,512] tile per sample; q/k via
    one M=128 matmul (k rows 0:64, q rows 64:128 with bias; q DMA-shifted
    to partitions 0:64 so S^T operands can row-pack the PE).
  - phase 2: 9 steps (7x 512-wide t-chunks + 2x 256 at the tail so the
    last finals chain is short). Per step, per pair of 128-wide s-chunks
    (one S^T/exp pair emitted ahead, crossing step boundaries):
      S^T halves into a 3-deep PSUM ring (fp16 row-packed matmuls)
      E = exp(S^T)       (ACT, per-half, PSUM -> SBUF bf16)
      E2 = Ea + Eb       (GpSimd)
      out[cc] += matmul(lhsT=v^T[s,c], rhs=E)   (bf16, 4 c-chunks)
    den work is an add-tree (E4/E8/E16 on DVE, bf16) + ONE ones-matmul
    per step, emitted inside the NEXT step so its operands are ready.
  - finals for step i run inside step i+1: oacc -> SBUF as bf16 (DVE,
    emitted before the tree tail so banks free in time), r = exp(-ln(den))
    on ACT (same table set as Exp; DVE reciprocal costs 4us),
    result = out0*r + xg -> one DMA out. Only the last (256-wide) step's
    finals are exposed; they split muls across DVE and GpSimd.
"""
import numpy as np
import ml_dtypes
import concourse.bass as bass
import concourse.bacc as bacc
import concourse.tile as tile
from concourse import mybir
from concourse.bass_utils import run_bass_kernel_spmd

F32 = mybir.dt.float32
FP16 = mybir.dt.float16
BF16 = mybir.dt.bfloat16
AF = mybir.ActivationFunctionType

B, C, T, D = 16, 512, 2048, 64
NCORES = 8
BPC = B // NCORES          # samples per core
CCH = C // 128             # 4 channel chunks
SCH = T // 128             # 16 s chunks
NPR = SCH // 2             # 8 s-chunk pairs

# phase-2 steps: (sample, t-offset, t-width)
STEPS = ([(0, tc * 512, 512) for tc in range(4)]
         + [(1, tc * 512, 512) for tc in range(3)]
         + [(1, 1536, 256), (1, 1792, 128), (1, 1920, 128)])

PROFILE = False            # set True before calling kernel() to capture HW time
LAST_EXEC_NS = None
_CACHE = {}


def _build():
    nc = bacc.Bacc("TRN2", target_bir_lowering=False, debug=False,
                   enable_asserts=False)
    xd = nc.dram_tensor("x", [BPC, 128, CCH, T], FP16,
                        kind="ExternalInput").ap()
    xgd = nc.dram_tensor("xg", [BPC, 128, CCH, T], F32,
                         kind="ExternalInput").ap()
    wkqT = nc.dram_tensor("wkqT", [C, 2 * D], FP16, kind="ExternalInput").ap()
    wvT = nc.dram_tensor("wvT", [C, C], FP16, kind="ExternalInput").ap()
    bqd = nc.dram_tensor("bq", [D, 1], F32, kind="ExternalInput").ap()
    onesd = nc.dram_tensor("ones", [128, 128], BF16, kind="ExternalInput").ap()
    m30d = nc.dram_tensor("m30", [128, 1], F32, kind="ExternalInput").ap()
    outd = nc.dram_tensor("out", [BPC, 128, CCH, T], F32,
                          kind="ExternalOutput").ap()

    with tile.TileContext(nc) as tc:
        with tc.tile_pool(name="const", bufs=1) as constp, \
             tc.tile_pool(name="xp", bufs=1) as xp, \
             tc.tile_pool(name="vtp", bufs=1) as vtp, \
             tc.tile_pool(name="qkp", bufs=1) as qkp, \
             tc.tile_pool(name="etp", bufs=1) as etp, \
             tc.tile_pool(name="finp", bufs=1) as finp, \
             tc.tile_pool(name="ps", bufs=1, space="PSUM") as ps:

            # ---- warmup source: memset (no DMA dependency), so the PE
            # can start spinning at t~0 while everything streams in
            warm_src = constp.tile([128, 128], BF16, name="wsrc", tag="wsrc")
            nc.gpsimd.memset(warm_src[:], 1.0)

            # ---- constants: all on the gpsimd queue (sync carries x)
            wv_big = constp.tile([128, CCH, C], FP16)
            nc.gpsimd.dma_start(
                out=wv_big, in_=wvT.rearrange("(c p) o -> p c o", p=128))
            wkq_big = constp.tile([128, CCH, 2 * D], FP16)
            nc.gpsimd.dma_start(
                out=wkq_big, in_=wkqT.rearrange("(c p) d -> p c d", p=128))
            ones = constp.tile([128, 128], BF16)
            nc.gpsimd.dma_start(out=ones, in_=onesd)
            bq_full = constp.tile([128, 1], F32)
            nc.gpsimd.dma_start(out=bq_full[D:2 * D, :], in_=bqd)
            bq_hi = bq_full[D:2 * D, :]
            m30 = constp.tile([128, 1], F32)
            nc.gpsimd.dma_start(out=m30, in_=m30d)

            # ---- x loads: sample 0 sliced on sync, sample 1 on scalar
            x_big_all = [xp.tile([128, CCH, T], FP16, name=f"x_{b}",
                                 tag=f"x{b}") for b in range(BPC)]
            x0_slices = [(0, 256), (256, 256), (512, 512), (1024, 512),
                         (1536, 512)]
            for off, w in x0_slices:
                nc.sync.dma_start(out=x_big_all[0][:, :, off:off + w],
                                  in_=xd[0, :, :, off:off + w])
            for q2 in range(2):
                qsl = slice(q2 * 1024, (q2 + 1) * 1024)
                nc.scalar.dma_start(out=x_big_all[1][:, :, qsl],
                                    in_=xd[1, :, :, qsl])
            wv_sb = [wv_big[:, cc, :] for cc in range(CCH)]
            wkq_sb = [wkq_big[:, cc, :] for cc in range(CCH)]
            x_sb_all = [[x_big_all[b][:, cc, :] for cc in range(CCH)]
                        for b in range(BPC)]

            # ---- warmup: keep the PE busy while x streams in, so the
            # HAM clock gate opens (~3.4us of activity) before real work
            warm = ps.tile([128, 512], F32, name="warm", tag="stp", bufs=3)
            for i in range(24):
                nc.tensor.matmul(warm[:, 0:128], warm_src, warm_src,
                                 start=(i == 0), stop=(i == 23))

            # ================= phase 1: v^T and q/k, both samples ========
            vt_all, q_all, k_all = {}, {}, {}
            qhi_all, khi_all = {}, {}
            et = {}       # (si, pr) -> (e_a, e_b)
            tree = {}     # si -> dict(e2s, e4, e8, e16)

            def emit_pair(si, pr):
                """Scores + exp for (si, pr), plus the den add-tree partials
                (gpsimd e2, DVE e4/e8/e16). Emitted one step ahead of the
                consuming out-matmuls, so all of it is off the critical
                path by construction."""
                b, toff, w = STEPS[si]
                st = tree.setdefault(si, {"e2": {}, "e4": [], "e8": []})
                halves = []
                for h in range(2):
                    sc = 2 * pr + h
                    stp_h = ps.tile([128, 512], F32,
                                    name=f"st_{si}_{pr}_{h}", tag="stp",
                                    bufs=3)
                    if h == 0:
                        lhsT = k_all[b][:, sc * 128:(sc + 1) * 128]
                        rhs = q_all[b][:, toff:toff + w]
                    else:
                        lhsT = khi_all[b][D:2 * D, sc * 128:(sc + 1) * 128]
                        rhs = qhi_all[b][D:2 * D, toff:toff + w]
                    nc.tensor.matmul(stp_h[:, :w], lhsT, rhs,
                                     start=True, stop=True)
                    halves.append(stp_h)
                es = []
                for h in range(2):
                    e_h = etp.tile([128, 512], BF16,
                                   name=f"et_{si}_{pr}_{h}", tag="et",
                                   bufs=32)
                    # constant shift cancels in softmax; keeps den in a
                    # range where downstream fp stays well-behaved
                    nc.scalar.activation(out=e_h[:, :w],
                                         in_=halves[h][:, :w], func=AF.Exp,
                                         bias=m30[:])
                    es.append(e_h)
                et[(si, pr)] = tuple(es)
                e2 = etp.tile([128, 512], BF16, name=f"e2_{si}_{pr}",
                              tag="e2", bufs=3)
                nc.gpsimd.tensor_add(e2[:, :w], es[0][:, :w], es[1][:, :w])
                st["e2"][pr] = e2
                if pr % 2 == 1:
                    t4 = etp.tile([128, 512], BF16,
                                  name=f"e4_{si}_{pr // 2}", tag="e4",
                                  bufs=2)
                    nc.vector.tensor_add(t4[:, :w],
                                         st["e2"].pop(pr - 1)[:, :w],
                                         st["e2"].pop(pr)[:, :w])
                    st["e4"].append(t4)
                if pr == 3 or pr == 7:
                    t8 = etp.tile([128, 512], BF16,
                                  name=f"e8_{si}_{pr // 4}", tag="e8",
                                  bufs=2)
                    nc.vector.tensor_add(t8[:, :w], st["e4"][-2][:, :w],
                                         st["e4"][-1][:, :w])
                    st["e8"].append(t8)
                if pr == 7:
                    e16 = etp.tile([128, 512], BF16, name=f"e16_{si}",
                                   tag="e16", bufs=2)
                    nc.vector.tensor_add(e16[:, :w], st["e8"][0][:, :w],
                                         st["e8"][1][:, :w])
                    st["e16"] = e16

            for b in range(BPC):
                x_sb = x_sb_all[b]

                # v^T tiles (bf16): vt[b][:, sc, o]
                vt_big = vtp.tile([128, SCH, C], BF16, name=f"vt_{b}",
                                  tag=f"vt{b}")
                for sc in range(SCH):
                    vps = ps.tile([128, 512], F32, name=f"vps_{b}_{sc}",
                                  tag=f"o{'AB'[sc % 2]}0")
                    for cc in range(CCH):
                        nc.tensor.matmul(
                            vps[:], x_sb[cc][:, sc * 128:(sc + 1) * 128],
                            wv_sb[cc][:],
                            start=(cc == 0), stop=(cc == CCH - 1))
                    nc.vector.tensor_copy(out=vt_big[:, sc, :], in_=vps[:])
                    if b == 1 and sc >= 8:
                        # spread step-0's scores/exp over sample 1's tail
                        emit_pair(0, sc - 8)
                vt_all[b] = vt_big

                # q, k via one M=128 matmul; q shifted to partitions 0:64,
                # k replicated to 64:128 so S^T pairs can row-pack the PE
                q_hi = qkp.tile([128, T], FP16, name=f"qh_{b}", tag=f"qh{b}")
                k_hi = qkp.tile([128, T], FP16, name=f"kh_{b}", tag=f"kh{b}")
                q_sb = qkp.tile([D, T], FP16, name=f"q_{b}", tag=f"q{b}")
                k_sb = qkp.tile([D, T], FP16, name=f"k_{b}", tag=f"k{b}")
                for tc_i in range(4):
                    tsl = slice(tc_i * 512, (tc_i + 1) * 512)
                    qps = ps.tile([128, 512], F32, name=f"qps_{b}_{tc_i}",
                                  tag=f"o{'AB'[tc_i % 2]}1")
                    for cc in range(CCH):
                        nc.tensor.matmul(qps[:], wkq_sb[cc][:],
                                         x_sb[cc][:, tsl],
                                         start=(cc == 0), stop=(cc == CCH - 1))
                    nc.vector.tensor_copy(out=k_sb[:, tsl], in_=qps[0:D, :])
                    nc.scalar.activation(out=q_hi[D:2 * D, tsl],
                                         in_=qps[D:2 * D, :],
                                         func=AF.Identity, bias=bq_hi[:],
                                         scale=1.0)
                    nc.gpsimd.dma_start(out=q_sb[:, tsl],
                                        in_=q_hi[D:2 * D, tsl])
                nc.gpsimd.dma_start(out=k_hi[D:2 * D, :], in_=k_sb[:, :])
                q_all[b], k_all[b] = q_sb, k_sb
                qhi_all[b], khi_all[b] = q_hi, k_hi

            # ================= phase 2: attention, two cc-half passes ====
            nsteps = len(STEPS)
            pend = None   # (si, ob, xg_t) awaiting recip+finals in step si+1

            def emit_finals(si, ob, xg_t, last=False):
                b, toff, w = STEPS[si]
                den_ps = tree[si]["den"]
                scr = finp.tile([128, 512], F32, name=f"sc_{si}", tag="scr",
                                bufs=2)
                recip = finp.tile([128, 512], F32, name=f"rc_{si}", tag="rc",
                                  bufs=2)
                nc.vector.reciprocal_approx_accurate(
                    out=recip[:, :w], in_=den_ps[:, :w], scratch=scr[:, :w])
                t_f = finp.tile([128, CCH, 512], F32, name=f"f_{si}",
                                tag="f", bufs=2)
                for cc in range(CCH):
                    eng = nc.gpsimd if (last and cc >= 2) else nc.vector
                    eng.tensor_mul(t_f[:, cc, :w], ob[:, cc, :w],
                                   recip[:, :w])
                    eng.tensor_add(t_f[:, cc, :w], t_f[:, cc, :w],
                                   xg_t[:, cc, :w])
                    if last:
                        nc.sync.dma_start(out=outd[b, :, cc, toff:toff + w],
                                          in_=t_f[:, cc, :w])
                if not last:
                    nc.sync.dma_start(out=outd[b, :, :, toff:toff + w],
                                      in_=t_f[:, :, :w])
                del tree[si]

            for si in range(nsteps):
                b, toff, w = STEPS[si]
                fut = si + 1 if si + 1 < nsteps else None
                last = si == nsteps - 1
                oacc = [ps.tile([128, 512], F32, name=f"o_{si}_{cc}",
                                tag=f"o{'AB'[cc // 2]}{cc % 2}")
                        for cc in range(CCH)]
                xg_t = finp.tile([128, CCH, 512], F32,
                                 name=f"xg_{si}", tag="xg", bufs=2)
                nc.sync.dma_start(out=xg_t[:, :, :w],
                                  in_=xgd[b, :, :, toff:toff + w])
                ob = finp.tile([128, CCH, 512], F32, name=f"ob_{si}",
                               tag="ob", bufs=2)

                def half_pass(ccs, prs_future):
                    fi = 0
                    for pr in range(NPR):
                        e_a, e_b = et[(si, pr)]
                        for h in range(2):
                            sc = 2 * pr + h
                            e_sl = (e_a if h == 0 else e_b)[:, :w]
                            for cc in ccs:
                                nc.tensor.matmul(
                                    oacc[cc][:, :w],
                                    vt_all[b][:, sc,
                                              cc * 128:(cc + 1) * 128],
                                    e_sl, start=(sc == 0),
                                    stop=(sc == SCH - 1))
                        if fut is not None and pr % 2 == 1:
                            emit_pair(fut, prs_future[fi])
                            fi += 1
                    for cc in ccs:
                        nc.vector.tensor_copy(out=ob[:, cc, :w],
                                              in_=oacc[cc][:, :w])

                # pass A (cc 0,1): also run the previous step's finals
                half_pass((0, 1), (0, 1, 2, 3))
                if pend is not None:
                    fsi, fob, fxg = pend
                    emit_finals(fsi, fob, fxg)
                    pend = None
                # den for THIS step (its E16 completed during step si-1)
                den_ps = ps.tile([128, 512], F32, name=f"den_{si}",
                                 tag="den", bufs=1)
                nc.tensor.matmul(den_ps[:, :w], ones,
                                 tree[si]["e16"][:, :w],
                                 start=True, stop=True)
                tree[si]["den"] = den_ps
                # pass B (cc 2,3)
                half_pass((2, 3), (4, 5, 6, 7))
                et_done = [et.pop((si, pr)) for pr in range(NPR)]
                del et_done
                if last:
                    emit_finals(si, ob, xg_t, last=True)
                else:
                    pend = (si, ob, xg_t)
    nc.compile()
    return nc


def _get_nc():
    if "nc" not in _CACHE:
        _CACHE["nc"] = _build()
    return _CACHE["nc"]


def kernel(x, wq, bq, wk, bk, wv, bv, gamma):
    global LAST_EXEC_NS
    g = float(np.asarray(gamma).reshape(-1)[0])
    x = np.asarray(x, np.float32)
    # fold gamma into the v path; bk cancels inside softmax; the v bias
    # contributes gamma*bv per channel (softmax rows sum to 1) -> fold it
    # plus the residual into xg
    wvT = np.ascontiguousarray(
        (g * np.asarray(wv, np.float32)).T).astype(np.float16)
    wkqT = np.concatenate([np.asarray(wk, np.float32).T,
                           np.asarray(wq, np.float32).T],
                          axis=1).astype(np.float16)
    bq2 = np.asarray(bq, np.float32).reshape(D, 1)
    gbv = (g * np.asarray(bv, np.float32)).reshape(1, C, 1)
    xg = x + gbv
    # device layout: [B, p, cc, T] with original c == cc*128 + p
    xg = np.ascontiguousarray(
        xg.reshape(B, CCH, 128, T).transpose(0, 2, 1, 3))
    ones = np.ones((128, 128), ml_dtypes.bfloat16)
    m30c = np.full((128, 1), -30.0, np.float32)
    xh = np.ascontiguousarray(
        x.reshape(B, CCH, 128, T).transpose(0, 2, 1, 3)).astype(np.float16)

    in_maps = []
    for core in range(NCORES):
        sl = slice(core * BPC, (core + 1) * BPC)
        in_maps.append({
            "x": xh[sl], "xg": xg[sl],
            "wkqT": wkqT, "wvT": wvT,
            "bq": bq2, "ones": ones, "m30": m30c,
        })

    nc = _get_nc()
    res = run_bass_kernel_spmd(nc, in_maps, core_ids=list(range(NCORES)),
                               trace=PROFILE)
    LAST_EXEC_NS = res.exec_time_ns
    out = np.empty((B, 128, CCH, T), np.float32)
    for core in range(NCORES):
        out[core * BPC:(core + 1) * BPC] = res.results[core]["out"]
    return np.ascontiguousarray(
        out.transpose(0, 2, 1, 3)).reshape(B, C, T)
